# revision 34
# baseline (speedup 1.0000x reference)
"""GNN message passing (GraphConvolution) on 8 TRN2 NeuronCores.

reference:
    support = x @ W                                   # [N, H]
    msgs    = support[edge_src] * edge_w[:, None]     # [E, H]
    agg     = segment_sum(msgs, edge_dst, N)          # [N, H]
    out     = relu(agg + b)

Strategy (dst-node 1D sharding; src-sharded support + AllGather):
  - Core c owns dst nodes [c*NPC, (c+1)*NPC).
  - Phase 1 is src-sharded: core c computes support rows for ITS x shard
    only (12500 rows, padded to 12800), then one AllGather builds the
    full 102400-row support table; phase-2 gathers slice it into 4 runs
    of 32768 rows so int16 gather indices suffice.
  - Host routes edges: per core, edges are bucketed by
    (dst subtile of 128, run) and sorted; each bucket is padded to a
    multiple of 128 "edges" (idx=0, w=0). Bucket sizes are maxed over
    cores so a single SPMD NEFF works for all 8 cores.
  - Device gathers support rows with gpsimd.dma_gather (int16 indices),
    spreading calls over 4 SWDGE queues so descriptor generation runs on
    all four Q7 core pairs concurrently (this was the v1 bottleneck:
    ~7.6ns/descriptor of Q7 time, serialized on one pair).
  - Weighted one-hot indicator S[e, d] = w_e * (dstloc_e == d) built on
    VectorE; each 128-edge chunk reduced with one TensorE matmul
    accumulating in PSUM: psum[h, d] += gathered[e, h]^T-contract-e S[e, d]
  - Epilogue: ScalarE relu(psum + b) -> outT [H, NPC] -> host transpose.
"""

import math
import os

import ml_dtypes
import numpy as np

import concourse.bass as bass
import concourse.mybir as mybir
import concourse.tile as tile
from concourse import bacc
from concourse.bass_utils import run_bass_kernel_spmd
from concourse.library_config import mlp as _mlp_lib

BF16 = ml_dtypes.bfloat16
SUB = 128  # dst nodes per PSUM column block (one-hot width)
PSUM_COLS = 512  # PSUM bank tile free dim = subtiles-per-supertile * SUB

N_CORES = 8
NPC = 12500  # dst nodes per core
SHARD_PAD = 12800  # src rows per core, padded (multiple of 512)
TBLK = SHARD_PAD // 2  # 6400 shard rows per all-gather table
TROWS = TBLK * N_CORES  # 51200 rows per all-gathered table
RUNROWS = TROWS // 2  # 25600 rows per gather run (int16-safe)
N_RUNS = 4  # 2 tables x 2 equal runs each
XBLK = 3200  # phase-1 row block (half a table shard)


def _ceil_div(a, b):
    return (a + b - 1) // b


def prepare(x, edge_src, edge_dst, edge_w, W, b):
    """Host-side sharding/routing. Returns (cfg, in_maps)."""
    n_nodes, nfeat = x.shape
    nhid = W.shape[1]
    assert n_nodes == N_CORES * NPC
    n_sub = _ceil_div(NPC, SUB)
    sps = PSUM_COLS // SUB  # subtiles per supertile
    n_super = _ceil_div(n_sub, sps)

    src = np.asarray(edge_src).astype(np.int64)
    dst = np.asarray(edge_dst).astype(np.int64)
    ew = np.asarray(edge_w).astype(np.float32)

    # src -> (table, position): two all-gathered tables of 51200 rows
    # (8 ranks x 6400 shard-rows), each split into 2 equal 25600-row runs
    # (run ids 0..3) so all gather calls are similar-sized and the 4 SWDGE
    # queues stay balanced.
    s_rank = src // NPC
    s_loc = src % NPC
    s_k = s_loc // TBLK  # which table (0/1)
    s_tpos = s_rank * TBLK + (s_loc % TBLK)  # row within table, < 51200
    s_run = s_k * 2 + (s_tpos // RUNROWS)  # global run id 0..3
    s_idx = s_tpos % RUNROWS

    core_of = dst // NPC
    per_core = []
    counts = np.zeros((N_CORES, n_sub, N_RUNS), np.int64)
    for c in range(N_CORES):
        m = core_of == c
        p_c = s_idx[m]
        t_c = s_run[m]
        d_c = dst[m] - c * NPC
        w_c = ew[m]
        sub_c = d_c >> 7
        key = sub_c * N_RUNS + t_c
        order = np.argsort(key, kind="stable")
        p_c, d_c, w_c, key = p_c[order], d_c[order], w_c[order], key[order]
        cnt = np.bincount(key, minlength=n_sub * N_RUNS).reshape(n_sub, N_RUNS)
        counts[c] = cnt
        seg_start = np.zeros(n_sub * N_RUNS + 1, np.int64)
        np.cumsum(cnt.reshape(-1), out=seg_start[1:])
        per_core.append((p_c, d_c, w_c, seg_start))

    g_tab = _ceil_div(counts.max(axis=0), 128).astype(np.int64)
    g_tab[:, 0] = np.maximum(g_tab[:, 0], 1)  # every subtile gets >=1 chunk

    # Static call/chunk structure, in device program order: (S, t, s).
    supers = []
    chunk_off = 0
    seen = np.zeros(n_sub, np.int64)  # chunks placed so far per subtile
    total = g_tab.sum(axis=1)  # total chunks per subtile
    n_calls = 0
    run_order = tuple(range(N_RUNS))
    for S in range(n_super):
        subs = list(range(S * sps, min(S * sps + sps, n_sub)))
        calls = []
        for ti, t in enumerate(run_order):
            n_chunks = int(sum(g_tab[s, t] for s in subs))
            if n_chunks == 0:
                continue
            sub_local = []
            for s in subs:
                for _ in range(int(g_tab[s, t])):
                    sub_local.append(s - S * sps)
                    seen[s] += 1
            calls.append(
                dict(
                    t=t,
                    q=(S + ti) % 4,
                    n_chunks=n_chunks,
                    chunk_off=chunk_off,
                    sub_local=sub_local,
                )
            )
            n_calls += 1
            chunk_off += n_chunks
        w_cols = min(PSUM_COLS, NPC - S * PSUM_COLS)
        supers.append(dict(calls=calls, w=w_cols))
    nchunk = int(chunk_off)
    e_pad = nchunk * 128

    # real (per-segment) edge offsets, following the same (S, t, s) order
    seg_edge_off = {}
    pos = 0
    for S in range(n_super):
        subs = list(range(S * sps, min(S * sps + sps, n_sub)))
        for t in run_order:
            for s in subs:
                if g_tab[s, t] == 0:
                    continue
                seg_edge_off[(s, t)] = pos
                pos += int(g_tab[s, t]) * 128
    assert pos == e_pad

    in_maps = []
    w_bf = np.ascontiguousarray(np.asarray(W, np.float32).astype(BF16))
    bias = np.asarray(b, np.float32).reshape(nhid, 1).copy()
    iota = np.tile(np.arange(SUB, dtype=np.float32).astype(BF16)[None, :], (128, 1))
    x_np = np.asarray(x, np.float32)
    for c in range(N_CORES):
        p_c, d_c, w_c, seg_start = per_core[c]
        idx_pad = np.zeros(e_pad, np.int16)
        dl_pad = np.zeros(e_pad, np.float32)
        ww_pad = np.zeros(e_pad, np.float32)
        for (s, t), off in seg_edge_off.items():
            k = s * N_RUNS + t
            a, bnd = seg_start[k], seg_start[k + 1]
            n = bnd - a
            if n == 0:
                continue
            idx_pad[off : off + n] = p_c[a:bnd].astype(np.int16)
            dl_pad[off : off + n] = (d_c[a:bnd] & (SUB - 1)).astype(np.float32)
            ww_pad[off : off + n] = w_c[a:bnd]
        xT_shard = np.zeros((nfeat, SHARD_PAD), BF16)
        xT_shard[:, :NPC] = x_np[c * NPC : (c + 1) * NPC, :].T.astype(BF16)
        in_maps.append(
            dict(
                xT=xT_shard,
                wmat=w_bf,
                bias=bias,
                iota=iota,
                idx=np.ascontiguousarray(np.tile(idx_pad.reshape(-1, 16).T, (8, 1))),
                # dl/ew duplicated in adjacent pairs so every DVE operand's
                # innermost axis is a step-1 2-elem run -> 2x_1P perf mode
                dstloc=np.ascontiguousarray(
                    np.repeat(dl_pad.reshape(-1, 128).T.astype(BF16)[:, :, None], 2, 2)
                ),
                ew=np.ascontiguousarray(
                    np.repeat(ww_pad.reshape(-1, 128).T.astype(BF16)[:, :, None], 2, 2)
                ),
            )
        )

    cfg = dict(
        nfeat=nfeat,
        nhid=nhid,
        n_sub=n_sub,
        n_super=n_super,
        nchunk=nchunk,
        e_pad=e_pad,
        supers=supers,
    )
    return cfg, in_maps


def build_bass(cfg):
    F, H = cfg["nfeat"], cfg["nhid"]
    KC = F // 128
    assert F % 128 == 0 and H == 128 and SHARD_PAD % XBLK == 0 and XBLK % 128 == 0
    n_queues = int(os.environ.get("GNN_QUEUES", "4"))

    nc = bacc.Bacc(
        "TRN2",
        target_bir_lowering=False,
        debug=False,
        enable_asserts=True,
        num_devices=N_CORES,
        num_swdge_queues=n_queues,
    )
    f32, bf16, i16 = mybir.dt.float32, mybir.dt.bfloat16, mybir.dt.int16
    xT = nc.dram_tensor("xT", [F, SHARD_PAD], bf16, kind="ExternalInput")
    wmat = nc.dram_tensor("wmat", [F, H], bf16, kind="ExternalInput")
    bias = nc.dram_tensor("bias", [H, 1], f32, kind="ExternalInput")
    iota = nc.dram_tensor("iota", [128, SUB], bf16, kind="ExternalInput")
    idx = nc.dram_tensor("idx", [128, cfg["e_pad"] // 16], i16, kind="ExternalInput")
    dstloc = nc.dram_tensor(
        "dstloc", [128, cfg["nchunk"], 2], bf16, kind="ExternalInput"
    )
    ew = nc.dram_tensor("ew", [128, cfg["nchunk"], 2], bf16, kind="ExternalInput")
    outT = nc.dram_tensor("outT", [H, NPC], f32, kind="ExternalOutput")

    AF = mybir.ActivationFunctionType
    with tile.TileContext(nc) as tc:
        with (
            tc.tile_pool(name="const", bufs=1) as cpool,
            tc.tile_pool(name="xt", bufs=2) as xpool,
            tc.tile_pool(name="sup", bufs=2) as spool,
            tc.tile_pool(name="gath", bufs=10) as gpool,
            tc.tile_pool(name="ind", bufs=8) as ipool,
            tc.tile_pool(name="meta", bufs=16) as mpool,
            tc.tile_pool(name="outb", bufs=2) as opool,
            tc.tile_pool(name="psum", bufs=8, space="PSUM") as ppool,
            tc.tile_pool(name="dram", bufs=1, space="DRAM") as dpool,
        ):
            nc.gpsimd.load_library(_mlp_lib)
            # Two half-shard tiles so each AllGather's dependency covers only
            # half of phase 1 -> AG0 issues at phase-1 halftime.
            shards = [
                dpool.tile([TBLK, H], bf16, name=f"shard{k}") for k in range(2)
            ]
            tables = [
                dpool.tile([TROWS, H], bf16, addr_space="Shared", name=f"table{k}")
                for k in range(2)
            ]

            w_sb = cpool.tile([128, KC, H], bf16)
            nc.sync.dma_start(
                out=w_sb[:], in_=wmat.ap().rearrange("(c k) h -> k c h", k=128)
            )
            bias_sb = cpool.tile([H, 1], f32)
            nc.sync.dma_start(out=bias_sb[:], in_=bias.ap())
            iota_sb = cpool.tile([128, SUB], bf16)
            nc.sync.dma_start(out=iota_sb[:], in_=iota.ap())

            # ---- phase 1: support_shard = x_shard @ W (bf16) ----
            for blk in range(SHARD_PAD // XBLK):
                xts = []
                for kc in range(KC):
                    xt = xpool.tile([128, XBLK], bf16, tag=f"xt{kc}")
                    nc.sync.dma_start(
                        out=xt[:],
                        in_=xT.ap()[
                            kc * 128 : (kc + 1) * 128, blk * XBLK : (blk + 1) * XBLK
                        ],
                    )
                    xts.append(xt)
                st = spool.tile([128, XBLK], bf16)
                for i in range(XBLK // 128):
                    col = i * 128
                    ps = ppool.tile([128, 128], f32, tag="agg_ps")
                    for kc in range(KC):
                        nc.tensor.matmul(
                            ps[:],
                            xts[kc][:, col : col + 128],
                            w_sb[:, kc, :],
                            start=(kc == 0),
                            stop=(kc == KC - 1),
                        )
                    nc.scalar.activation(
                        out=st[:, col : col + 128], in_=ps[:], func=AF.Copy
                    )
                r0 = blk * XBLK - (blk // 2) * TBLK  # row offset within shard
                nc.sync.dma_start(
                    out=shards[blk // 2][r0 : r0 + XBLK, :].rearrange(
                        "(i p) h -> p i h", p=128
                    ),
                    in_=st[:].rearrange("p (i h) -> p i h", h=H),
                )
                if blk % 2 == 1:
                    # all-gather this half-shard as soon as it is stored
                    k = blk // 2
                    nc.gpsimd.collective_compute(
                        "AllGather",
                        mybir.AluOpType.bypass,
                        replica_groups=[list(range(N_CORES))],
                        ins=[shards[k].opt()],
                        outs=[tables[k].opt()],
                    )

            # ---- phase 2: gather + weighted-one-hot matmul segment sum ----
            for S, sup in enumerate(cfg["supers"]):
                # One PSUM bank per supertile: start=True on its first matmul
                # clears the whole bank; later matmuls overwrite-where-unwritten
                # / accumulate-where-written, so the 4 SUB-wide column regions
                # accumulate independently within the same bank.
                pss = ppool.tile([128, PSUM_COLS], f32, name="agg_ps", tag="agg_ps")
                total_chunks = sum(call["n_chunks"] for call in sup["calls"])
                mm_i = 0
                for call in sup["calls"]:
                    Gc = call["n_chunks"]
                    L = Gc * 128
                    idxt = mpool.tile([128, L // 16], i16, tag="idx")
                    c0 = call["chunk_off"] * 8  # idx plane col = chunk_off*128/16
                    nc.scalar.dma_start(
                        out=idxt[:], in_=idx.ap()[:, c0 : c0 + L // 16]
                    )
                    dlt = mpool.tile([128, Gc, 2], bf16, tag="dl")
                    nc.scalar.dma_start(
                        out=dlt[:],
                        in_=dstloc.ap()[
                            :, call["chunk_off"] : call["chunk_off"] + Gc, :
                        ],
                    )
                    ewt = mpool.tile([128, Gc, 2], bf16, tag="ew")
                    nc.scalar.dma_start(
                        out=ewt[:],
                        in_=ew.ap()[:, call["chunk_off"] : call["chunk_off"] + Gc, :],
                    )
                    t = call["t"]
                    row0 = (t % 2) * RUNROWS
                    src_ap = tables[t // 2][row0 : row0 + RUNROWS, :]
                    gt = gpool.tile([128, Gc, H], bf16)
                    nc.gpsimd.dma_gather(
                        gt[:],
                        src_ap,
                        idxt[:],
                        L,
                        L,
                        H,
                        single_packet=False,
                        queue_num=call["q"] % n_queues,
                    )
                    ind = ipool.tile([128, Gc, SUB], bf16)
                    ind4 = ind[:].rearrange("p g (a b) -> p g a b", b=2)
                    nc.vector.tensor_tensor(
                        out=ind4,
                        in0=iota_sb[:]
                        .rearrange("p (a b) -> p a b", b=2)[:, None, :, :]
                        .to_broadcast([128, Gc, SUB // 2, 2]),
                        in1=dlt[:][:, :, None, :].to_broadcast([128, Gc, SUB // 2, 2]),
                        op=mybir.AluOpType.is_equal,
                    )
                    nc.vector.tensor_tensor(
                        out=ind4,
                        in0=ind4,
                        in1=ewt[:][:, :, None, :].to_broadcast([128, Gc, SUB // 2, 2]),
                        op=mybir.AluOpType.mult,
                    )
                    for j in range(Gc):
                        so = call["sub_local"][j]
                        nc.tensor.matmul(
                            pss[:, so * SUB : (so + 1) * SUB],
                            gt[:, j, :],
                            ind[:, j, :],
                            start=(mm_i == 0),
                            stop=(mm_i == total_chunks - 1),
                        )
                        mm_i += 1
                ob = opool.tile([H, PSUM_COLS], f32)
                wS = sup["w"]
                # relu(psum + b) on DVE, keeping the ACT stream free to issue
                # metadata-prefetch DMAs without stalling behind PSUM waits
                nc.vector.tensor_scalar(
                    out=ob[:, :wS],
                    in0=pss[:, :wS],
                    scalar1=bias_sb[:],
                    scalar2=0.0,
                    op0=mybir.AluOpType.add,
                    op1=mybir.AluOpType.max,
                )
                nc.sync.dma_start(
                    out=outT.ap()[:, S * PSUM_COLS : S * PSUM_COLS + wS],
                    in_=ob[:, :wS],
                )
    nc.compile()
    return nc


def kernel(x, edge_src, edge_dst, edge_w, W, b):
    x = np.asarray(x)
    cfg, in_maps = prepare(x, edge_src, edge_dst, edge_w, W, b)
    nc = build_bass(cfg)
    want_trace = bool(int(os.environ.get("GNN_TRACE", "0")))
    core_ids = list(range(N_CORES))
    if want_trace:
        try:
            res = run_bass_kernel_spmd(nc, in_maps, core_ids=core_ids, trace=True)
        except Exception as e:
            print(f"traced run failed ({e}); retrying without trace")
            res = run_bass_kernel_spmd(nc, in_maps, core_ids=core_ids, trace=False)
    else:
        res = run_bass_kernel_spmd(nc, in_maps, core_ids=core_ids, trace=False)
    kernel.last_result = res
    out = np.concatenate([r["outT"].T for r in res.results], axis=0)
    return np.ascontiguousarray(out).astype(np.float32)


kernel.last_result = None


# revision 36
# speedup vs baseline: 1.0301x; 1.0301x over previous
"""GNN message passing (GraphConvolution) on 8 TRN2 NeuronCores.

reference:
    support = x @ W                                   # [N, H]
    msgs    = support[edge_src] * edge_w[:, None]     # [E, H]
    agg     = segment_sum(msgs, edge_dst, N)          # [N, H]
    out     = relu(agg + b)

Strategy (dst-node 1D sharding; src-sharded support + AllGather):
  - Core c owns dst nodes [c*NPC, (c+1)*NPC).
  - Phase 1 is src-sharded: core c computes support rows for ITS x shard
    only (12500 rows, padded to 12800), then one AllGather builds the
    full 102400-row support table; phase-2 gathers slice it into 4 runs
    of 32768 rows so int16 gather indices suffice.
  - Host routes edges: per core, edges are bucketed by
    (dst subtile of 128, run) and sorted; each bucket is padded to a
    multiple of 128 "edges" (idx=0, w=0). Bucket sizes are maxed over
    cores so a single SPMD NEFF works for all 8 cores.
  - Device gathers support rows with gpsimd.dma_gather (int16 indices),
    spreading calls over 4 SWDGE queues so descriptor generation runs on
    all four Q7 core pairs concurrently (this was the v1 bottleneck:
    ~7.6ns/descriptor of Q7 time, serialized on one pair).
  - Weighted one-hot indicator S[e, d] = w_e * (dstloc_e == d) built on
    VectorE; each 128-edge chunk reduced with one TensorE matmul
    accumulating in PSUM: psum[h, d] += gathered[e, h]^T-contract-e S[e, d]
  - Epilogue: ScalarE relu(psum + b) -> outT [H, NPC] -> host transpose.
"""

import math
import os

import ml_dtypes
import numpy as np

import concourse.bass as bass
import concourse.mybir as mybir
import concourse.tile as tile
from concourse import bacc
from concourse.bass_utils import run_bass_kernel_spmd
from concourse.library_config import mlp as _mlp_lib

BF16 = ml_dtypes.bfloat16
SUB = 128  # dst nodes per PSUM column block (one-hot width)
PSUM_COLS = 512  # PSUM bank tile free dim = subtiles-per-supertile * SUB

N_CORES = 8
NPC = 12500  # dst nodes per core
SHARD_PAD = 12800  # src rows per core, padded (multiple of 512)
TBLK = SHARD_PAD // 2  # 6400 shard rows per all-gather table
TROWS = TBLK * N_CORES  # 51200 rows per all-gathered table
RUNROWS = TROWS // 2  # 25600 rows per gather run (int16-safe)
N_RUNS = 4  # 2 tables x 2 equal runs each
XBLK = 3200  # phase-1 row block (half a table shard)


def _ceil_div(a, b):
    return (a + b - 1) // b


def prepare(x, edge_src, edge_dst, edge_w, W, b):
    """Host-side sharding/routing. Returns (cfg, in_maps)."""
    n_nodes, nfeat = x.shape
    nhid = W.shape[1]
    assert n_nodes == N_CORES * NPC
    n_sub = _ceil_div(NPC, SUB)
    sps = PSUM_COLS // SUB  # subtiles per supertile
    n_super = _ceil_div(n_sub, sps)

    src = np.asarray(edge_src).astype(np.int64)
    dst = np.asarray(edge_dst).astype(np.int64)
    ew = np.asarray(edge_w).astype(np.float32)

    # src -> (table, position): two all-gathered tables of 51200 rows
    # (8 ranks x 6400 shard-rows), each split into 2 equal 25600-row runs
    # (run ids 0..3) so all gather calls are similar-sized and the 4 SWDGE
    # queues stay balanced.
    s_rank = src // NPC
    s_loc = src % NPC
    s_k = s_loc // TBLK  # which table (0/1)
    s_tpos = s_rank * TBLK + (s_loc % TBLK)  # row within table, < 51200
    s_run = s_k * 2 + (s_tpos // RUNROWS)  # global run id 0..3
    s_idx = s_tpos % RUNROWS

    core_of = dst // NPC
    per_core = []
    counts = np.zeros((N_CORES, n_sub, N_RUNS), np.int64)
    for c in range(N_CORES):
        m = core_of == c
        p_c = s_idx[m]
        t_c = s_run[m]
        d_c = dst[m] - c * NPC
        w_c = ew[m]
        sub_c = d_c >> 7
        key = sub_c * N_RUNS + t_c
        order = np.argsort(key, kind="stable")
        p_c, d_c, w_c, key = p_c[order], d_c[order], w_c[order], key[order]
        cnt = np.bincount(key, minlength=n_sub * N_RUNS).reshape(n_sub, N_RUNS)
        counts[c] = cnt
        seg_start = np.zeros(n_sub * N_RUNS + 1, np.int64)
        np.cumsum(cnt.reshape(-1), out=seg_start[1:])
        per_core.append((p_c, d_c, w_c, seg_start))

    g_tab = _ceil_div(counts.max(axis=0), 128).astype(np.int64)
    g_tab[:, 0] = np.maximum(g_tab[:, 0], 1)  # every subtile gets >=1 chunk

    # Static call/chunk structure, in device program order: (S, t, s).
    supers = []
    chunk_off = 0
    seen = np.zeros(n_sub, np.int64)  # chunks placed so far per subtile
    total = g_tab.sum(axis=1)  # total chunks per subtile
    n_calls = 0
    run_order = tuple(range(N_RUNS))
    for S in range(n_super):
        subs = list(range(S * sps, min(S * sps + sps, n_sub)))
        calls = []
        for ti, t in enumerate(run_order):
            n_chunks = int(sum(g_tab[s, t] for s in subs))
            if n_chunks == 0:
                continue
            sub_local = []
            for s in subs:
                for _ in range(int(g_tab[s, t])):
                    sub_local.append(s - S * sps)
                    seen[s] += 1
            calls.append(
                dict(
                    t=t,
                    q=(S + ti) % 4,
                    n_chunks=n_chunks,
                    chunk_off=chunk_off,
                    sub_local=sub_local,
                )
            )
            n_calls += 1
            chunk_off += n_chunks
        w_cols = min(PSUM_COLS, NPC - S * PSUM_COLS)
        supers.append(dict(calls=calls, w=w_cols))
    nchunk = int(chunk_off)
    e_pad = nchunk * 128

    # real (per-segment) edge offsets, following the same (S, t, s) order
    seg_edge_off = {}
    pos = 0
    for S in range(n_super):
        subs = list(range(S * sps, min(S * sps + sps, n_sub)))
        for t in run_order:
            for s in subs:
                if g_tab[s, t] == 0:
                    continue
                seg_edge_off[(s, t)] = pos
                pos += int(g_tab[s, t]) * 128
    assert pos == e_pad

    in_maps = []
    w_bf = np.ascontiguousarray(np.asarray(W, np.float32).astype(BF16))
    bias = np.asarray(b, np.float32).reshape(nhid, 1).copy()
    iota = np.tile(np.arange(SUB, dtype=np.float32).astype(BF16)[None, :], (128, 1))
    x_np = np.asarray(x, np.float32)
    for c in range(N_CORES):
        p_c, d_c, w_c, seg_start = per_core[c]
        idx_pad = np.zeros(e_pad, np.int16)
        dl_pad = np.zeros(e_pad, np.float32)
        ww_pad = np.zeros(e_pad, np.float32)
        for (s, t), off in seg_edge_off.items():
            k = s * N_RUNS + t
            a, bnd = seg_start[k], seg_start[k + 1]
            n = bnd - a
            if n == 0:
                continue
            idx_pad[off : off + n] = p_c[a:bnd].astype(np.int16)
            dl_pad[off : off + n] = (d_c[a:bnd] & (SUB - 1)).astype(np.float32)
            ww_pad[off : off + n] = w_c[a:bnd]
        xT_shard = np.zeros((nfeat, SHARD_PAD), BF16)
        xT_shard[:, :NPC] = x_np[c * NPC : (c + 1) * NPC, :].T.astype(BF16)
        in_maps.append(
            dict(
                xT=xT_shard,
                wmat=w_bf,
                bias=bias,
                iota=iota,
                idx=np.ascontiguousarray(np.tile(idx_pad.reshape(-1, 16).T, (8, 1))),
                # dl/ew duplicated in adjacent pairs so every DVE operand's
                # innermost axis is a step-1 2-elem run -> 2x_1P perf mode
                dstloc=np.ascontiguousarray(
                    np.repeat(dl_pad.reshape(-1, 128).T.astype(BF16)[:, :, None], 2, 2)
                ),
                ew=np.ascontiguousarray(
                    np.repeat(ww_pad.reshape(-1, 128).T.astype(BF16)[:, :, None], 2, 2)
                ),
            )
        )

    cfg = dict(
        nfeat=nfeat,
        nhid=nhid,
        n_sub=n_sub,
        n_super=n_super,
        nchunk=nchunk,
        e_pad=e_pad,
        supers=supers,
    )
    return cfg, in_maps


def build_bass(cfg):
    F, H = cfg["nfeat"], cfg["nhid"]
    KC = F // 128
    assert F % 128 == 0 and H == 128 and SHARD_PAD % XBLK == 0 and XBLK % 128 == 0
    n_queues = int(os.environ.get("GNN_QUEUES", "4"))

    nc = bacc.Bacc(
        "TRN2",
        target_bir_lowering=False,
        debug=False,
        enable_asserts=True,
        num_devices=N_CORES,
        num_swdge_queues=n_queues,
    )
    f32, bf16, i16 = mybir.dt.float32, mybir.dt.bfloat16, mybir.dt.int16
    xT = nc.dram_tensor("xT", [F, SHARD_PAD], bf16, kind="ExternalInput")
    wmat = nc.dram_tensor("wmat", [F, H], bf16, kind="ExternalInput")
    bias = nc.dram_tensor("bias", [H, 1], f32, kind="ExternalInput")
    iota = nc.dram_tensor("iota", [128, SUB], bf16, kind="ExternalInput")
    idx = nc.dram_tensor("idx", [128, cfg["e_pad"] // 16], i16, kind="ExternalInput")
    dstloc = nc.dram_tensor(
        "dstloc", [128, cfg["nchunk"], 2], bf16, kind="ExternalInput"
    )
    ew = nc.dram_tensor("ew", [128, cfg["nchunk"], 2], bf16, kind="ExternalInput")
    outT = nc.dram_tensor("outT", [H, NPC], f32, kind="ExternalOutput")

    AF = mybir.ActivationFunctionType
    with tile.TileContext(nc) as tc:
        with (
            tc.tile_pool(name="const", bufs=1) as cpool,
            tc.tile_pool(name="xt", bufs=2) as xpool,
            tc.tile_pool(name="sup", bufs=2) as spool,
            tc.tile_pool(name="gath", bufs=12) as gpool,
            tc.tile_pool(name="ind", bufs=10) as ipool,
            tc.tile_pool(name="meta", bufs=20) as mpool,
            tc.tile_pool(name="outb", bufs=2) as opool,
            tc.tile_pool(name="psum", bufs=8, space="PSUM") as ppool,
            tc.tile_pool(name="dram", bufs=1, space="DRAM") as dpool,
        ):
            nc.gpsimd.load_library(_mlp_lib)
            # Two half-shard tiles so each AllGather's dependency covers only
            # half of phase 1 -> AG0 issues at phase-1 halftime.
            shards = [
                dpool.tile([TBLK, H], bf16, name=f"shard{k}") for k in range(2)
            ]
            tables = [
                dpool.tile([TROWS, H], bf16, addr_space="Shared", name=f"table{k}")
                for k in range(2)
            ]

            w_sb = cpool.tile([128, KC, H], bf16)
            nc.sync.dma_start(
                out=w_sb[:], in_=wmat.ap().rearrange("(c k) h -> k c h", k=128)
            )
            bias_sb = cpool.tile([H, 1], f32)
            nc.sync.dma_start(out=bias_sb[:], in_=bias.ap())
            iota_sb = cpool.tile([128, SUB], bf16)
            nc.sync.dma_start(out=iota_sb[:], in_=iota.ap())

            # ---- phase 1: support_shard = x_shard @ W (bf16) ----
            for blk in range(SHARD_PAD // XBLK):
                xts = []
                for kc in range(KC):
                    xt = xpool.tile([128, XBLK], bf16, tag=f"xt{kc}")
                    nc.sync.dma_start(
                        out=xt[:],
                        in_=xT.ap()[
                            kc * 128 : (kc + 1) * 128, blk * XBLK : (blk + 1) * XBLK
                        ],
                    )
                    xts.append(xt)
                st = spool.tile([128, XBLK], bf16)
                for i in range(XBLK // 128):
                    col = i * 128
                    ps = ppool.tile([128, 128], f32, tag="agg_ps")
                    for kc in range(KC):
                        nc.tensor.matmul(
                            ps[:],
                            xts[kc][:, col : col + 128],
                            w_sb[:, kc, :],
                            start=(kc == 0),
                            stop=(kc == KC - 1),
                        )
                    nc.scalar.activation(
                        out=st[:, col : col + 128], in_=ps[:], func=AF.Copy
                    )
                r0 = blk * XBLK - (blk // 2) * TBLK  # row offset within shard
                nc.sync.dma_start(
                    out=shards[blk // 2][r0 : r0 + XBLK, :].rearrange(
                        "(i p) h -> p i h", p=128
                    ),
                    in_=st[:].rearrange("p (i h) -> p i h", h=H),
                )
                if blk % 2 == 1:
                    # all-gather this half-shard as soon as it is stored
                    k = blk // 2
                    nc.gpsimd.collective_compute(
                        "AllGather",
                        mybir.AluOpType.bypass,
                        replica_groups=[list(range(N_CORES))],
                        ins=[shards[k].opt()],
                        outs=[tables[k].opt()],
                    )

            # ---- phase 2: gather + weighted-one-hot matmul segment sum ----
            for S, sup in enumerate(cfg["supers"]):
                # One PSUM bank per supertile: start=True on its first matmul
                # clears the whole bank; later matmuls overwrite-where-unwritten
                # / accumulate-where-written, so the 4 SUB-wide column regions
                # accumulate independently within the same bank.
                pss = ppool.tile([128, PSUM_COLS], f32, name="agg_ps", tag="agg_ps")
                total_chunks = sum(call["n_chunks"] for call in sup["calls"])
                mm_i = 0
                for call in sup["calls"]:
                    Gc = call["n_chunks"]
                    L = Gc * 128
                    idxt = mpool.tile([128, L // 16], i16, tag="idx")
                    c0 = call["chunk_off"] * 8  # idx plane col = chunk_off*128/16
                    nc.scalar.dma_start(
                        out=idxt[:], in_=idx.ap()[:, c0 : c0 + L // 16]
                    )
                    dlt = mpool.tile([128, Gc, 2], bf16, tag="dl")
                    nc.scalar.dma_start(
                        out=dlt[:],
                        in_=dstloc.ap()[
                            :, call["chunk_off"] : call["chunk_off"] + Gc, :
                        ],
                    )
                    ewt = mpool.tile([128, Gc, 2], bf16, tag="ew")
                    nc.scalar.dma_start(
                        out=ewt[:],
                        in_=ew.ap()[:, call["chunk_off"] : call["chunk_off"] + Gc, :],
                    )
                    t = call["t"]
                    row0 = (t % 2) * RUNROWS
                    src_ap = tables[t // 2][row0 : row0 + RUNROWS, :]
                    gt = gpool.tile([128, Gc, H], bf16)
                    nc.gpsimd.dma_gather(
                        gt[:],
                        src_ap,
                        idxt[:],
                        L,
                        L,
                        H,
                        single_packet=False,
                        queue_num=call["q"] % n_queues,
                    )
                    ind = ipool.tile([128, Gc, SUB], bf16)
                    ind4 = ind[:].rearrange("p g (a b) -> p g a b", b=2)
                    nc.vector.tensor_tensor(
                        out=ind4,
                        in0=iota_sb[:]
                        .rearrange("p (a b) -> p a b", b=2)[:, None, :, :]
                        .to_broadcast([128, Gc, SUB // 2, 2]),
                        in1=dlt[:][:, :, None, :].to_broadcast([128, Gc, SUB // 2, 2]),
                        op=mybir.AluOpType.is_equal,
                    )
                    nc.vector.tensor_tensor(
                        out=ind4,
                        in0=ind4,
                        in1=ewt[:][:, :, None, :].to_broadcast([128, Gc, SUB // 2, 2]),
                        op=mybir.AluOpType.mult,
                    )
                    for j in range(Gc):
                        so = call["sub_local"][j]
                        nc.tensor.matmul(
                            pss[:, so * SUB : (so + 1) * SUB],
                            gt[:, j, :],
                            ind[:, j, :],
                            start=(mm_i == 0),
                            stop=(mm_i == total_chunks - 1),
                        )
                        mm_i += 1
                ob = opool.tile([H, PSUM_COLS], f32)
                wS = sup["w"]
                nc.scalar.activation(
                    out=ob[:, :wS],
                    in_=pss[:, :wS],
                    func=AF.Relu,
                    bias=bias_sb[:],
                    scale=1.0,
                )
                nc.sync.dma_start(
                    out=outT.ap()[:, S * PSUM_COLS : S * PSUM_COLS + wS],
                    in_=ob[:, :wS],
                )
    nc.compile()
    return nc


def kernel(x, edge_src, edge_dst, edge_w, W, b):
    x = np.asarray(x)
    cfg, in_maps = prepare(x, edge_src, edge_dst, edge_w, W, b)
    nc = build_bass(cfg)
    want_trace = bool(int(os.environ.get("GNN_TRACE", "0")))
    core_ids = list(range(N_CORES))
    if want_trace:
        try:
            res = run_bass_kernel_spmd(nc, in_maps, core_ids=core_ids, trace=True)
        except Exception as e:
            print(f"traced run failed ({e}); retrying without trace")
            res = run_bass_kernel_spmd(nc, in_maps, core_ids=core_ids, trace=False)
    else:
        res = run_bass_kernel_spmd(nc, in_maps, core_ids=core_ids, trace=False)
    kernel.last_result = res
    out = np.concatenate([r["outT"].T for r in res.results], axis=0)
    return np.ascontiguousarray(out).astype(np.float32)


kernel.last_result = None
